# revision 13
# baseline (speedup 1.0000x reference)
"""Trainium2 Bass kernel for nn_Attention_C_12111807775306.

Structure exploited: attention_ca's output feeds ONLY the top-k expert
selection (batch element 0 alone); the expert conv branches apply to the
original input I. The first channel-attention runs on the host (cheap numpy,
batch 0 only) to pick the 4 experts; the device kernel computes: 4 expert
convs on I -> concat -> 3x3 conv (768->192) -> kv/q convs -> channel
attention -> 1x1 proj.

Sharding: 8 cores = 4 batches x 2 row-halves. Each core holds a 40-row slab
of its batch's (zero-padded) input and computes its 32 output rows plus halo
rows of the intermediate feature maps; the only cross-core communication is
one 26KB paired AllReduce carrying the q/k norm partial sums and the
per-head Gram partial sums. The v-convs are emitted after the AllReduce
trigger so the collective latency hides under PE work.

The 192-channel contraction splits into a 128-chunk and a 64-chunk; 64-chunk
matmuls cost the same as 128 ones, so the host ships duplicated copies of
the channel remainder ([b;b] and [b;b shifted one col]) letting two conv
shifts or two depthwise experts share one full 128-contraction matmul.
Matmul ordering is weight-stationary across row-groups so LDWEIGHTS
amortizes. Compute dtype bf16 (fp32 PSUM), fp32 output.
"""
import sys
sys.path.insert(0, "/opt/trn_rl_repo")
import numpy as np
import ml_dtypes

DIM = 192
HEADS = 6
B = 4
H = 64
W = 64
L = H * W
TOPK = 4
PADS = [0, 1, 2] * 4
KSZ = [1, 3, 5] * 4
GROUPS = [1] * 6 + [DIM] * 6
BF16 = ml_dtypes.bfloat16

SR = 40            # slab rows per core (4 pad + 32 out + 4 halo)
SC = 72            # slab cols (4 + 64 + 4)
LCORE = 32 * 64    # per-core attention length

G_E = [(2, 8), (10, 8), (18, 8), (26, 8), (34, 4)]   # expert outs rows [2,38)
G_F = [(3, 8), (11, 8), (19, 8), (27, 8), (35, 2)]   # fmap2 rows [3,37)
G_K = [(3, 7), (10, 7), (17, 7), (24, 7), (31, 6)]   # kvpre rows [3,37)

CH = [(0, 128), (128, 64)]

# vecs columns
VC_BIAS = 0        # ..3: expert slot biases (channels; used for A-chunk evac)
VC_XB = 4          # ex_out bias
VC_TEMP = 5
VC_MT = 6          # top mask (0 when rows 2..4 are outside the image)
VC_MB = 7          # bottom mask
VC_BIAS_MT = 8     # ..11
VC_BIAS_MB = 12    # ..15
VC_XB_MT = 16
VC_XB_MB = 17
VC_DW1 = 18        # 6 cols per dw1x1 slot (w, b, w*mt, b*mt, w*mb, b*mb)
VC_N = 32


def _l2n(x):
    return x / np.maximum(np.linalg.norm(x, axis=-1, keepdims=True), 1e-12)


def _select_experts(I, T, ca1_proj_w):
    """Replicate attention_ca + binning for batch 0 only; return top-4 idx."""
    b0I = I[0].astype(np.float64)
    b0T = T[0].astype(np.float64)
    pooled = b0T.reshape(DIM // 4, 4, L).mean(1)
    q = _l2n(b0I.reshape(HEADS, DIM // HEADS, L))
    k = _l2n(pooled.reshape(HEADS, 8, L))
    kt = np.tile(k, (1, 4, 1))
    s = np.einsum("hcl,hdl->hcd", q, kt)
    s = s - s.max(-1, keepdims=True)
    e = np.exp(s)
    attn = e / e.sum(-1, keepdims=True)
    out = np.einsum("hcd,hdl->hcl", attn, kt).reshape(DIM, H, W)
    fmap0 = np.einsum("oi,ihw->ohw", ca1_proj_w[:, :, 0, 0].astype(np.float64), out)
    m = fmap0.mean(axis=(0, 1))
    bins = np.array([m[(i * W) // 12: -(-((i + 1) * W) // 12)].mean()
                     for i in range(12)])
    return [int(x) for x in np.argsort(-bins, kind="stable")[:TOPK]]


def _b_pair_sets(ks):
    """Pairs of dx shifts sharing one 128-contraction matmul on [b; b<<1]."""
    sets = []
    for dy in range(ks):
        dx = 0
        while dx < ks:
            if dx + 1 < ks:
                sets.append((dy, dx, dx + 1))
                dx += 2
            else:
                sets.append((dy, dx, None))
                dx += 1
    return sets


def _dw_pairs(sel):
    """(j, ks) for packable dw pairs (slots 2j, 2j+1 both dw>1, same ksz)."""
    out = []
    for j in range(2):
        s0, s1 = 2 * j, 2 * j + 1
        if (GROUPS[sel[s0]] == DIM and KSZ[sel[s0]] > 1
                and GROUPS[sel[s1]] == DIM and KSZ[sel[s1]] > 1
                and KSZ[sel[s0]] == KSZ[sel[s1]]):
            out.append((j, KSZ[sel[s0]]))
    return out


def _build_and_run(sel, host_inputs):
    import concourse.mybir as mybir
    import concourse.tile as tile
    from concourse import bacc
    from concourse.bass_utils import run_bass_kernel_spmd

    bf = mybir.dt.bfloat16
    f32 = mybir.dt.float32
    AF = mybir.ActivationFunctionType
    ALU = mybir.AluOpType
    NCORES = 8
    RG = [[0, 1], [2, 3], [4, 5], [6, 7]]

    dense_slots = [i for i in range(TOPK) if GROUPS[sel[i]] == 1]
    dw1_slots = [i for i in range(TOPK) if GROUPS[sel[i]] == DIM and KSZ[sel[i]] == 1]
    pairs = _dw_pairs(sel)
    paired = {2 * j for (j, _) in pairs} | {2 * j + 1 for (j, _) in pairs}
    dw_solo = [i for i in range(TOPK)
               if GROUPS[sel[i]] == DIM and KSZ[sel[i]] > 1 and i not in paired]

    nc = bacc.Bacc("TRN2", target_bir_lowering=False, debug=False,
                   enable_asserts=False, num_devices=NCORES)

    # ---- DRAM I/O ----
    xin_d = nc.dram_tensor("xin", [DIM, SR, SC], bf, kind="ExternalInput")
    xb2s_d = (nc.dram_tensor("xb2s", [128, SR, SC], bf, kind="ExternalInput")
              if dense_slots else None)
    xb2_d = (nc.dram_tensor("xb2", [128, SR, SC], bf, kind="ExternalInput")
             if pairs else None)
    densea_d = {}
    densep_d = {}
    for i in dense_slots:
        kk = KSZ[sel[i]] ** 2
        ns = len(_b_pair_sets(KSZ[sel[i]]))
        densea_d[i] = nc.dram_tensor(f"e{i}_wa", [128, kk * DIM], bf,
                                     kind="ExternalInput")
        densep_d[i] = nc.dram_tensor(f"e{i}_wp", [128, ns * DIM], bf,
                                     kind="ExternalInput")
    diag_d = {}
    for i in dw_solo + [2 * j for (j, _) in pairs] + [2 * j + 1 for (j, _) in pairs]:
        kk = KSZ[sel[i]] ** 2
        diag_d[i] = nc.dram_tensor(f"e{i}_da", [128, kk * 128], bf,
                                   kind="ExternalInput")
    diagb_d = {i: nc.dram_tensor(f"e{i}_db", [64, KSZ[sel[i]] ** 2 * 64], bf,
                                 kind="ExternalInput") for i in dw_solo}
    pbdiag_d = {j: nc.dram_tensor(f"e_pb{j}", [128, ks * ks * 128], bf,
                                  kind="ExternalInput") for (j, ks) in pairs}
    wexout_d = nc.dram_tensor("wexout", [128, 6 * 9 * DIM], bf,
                              kind="ExternalInput")
    dkv0_d = nc.dram_tensor("dkv0", [128, 9 * 128], bf, kind="ExternalInput")
    dkv2_d = nc.dram_tensor("dkv2", [128, 9 * 128], bf, kind="ExternalInput")
    dkv1v_d = nc.dram_tensor("dkv1v", [64, 9 * 64], bf, kind="ExternalInput")
    dqkb_d = nc.dram_tensor("dqkb", [128, 9 * 128], bf, kind="ExternalInput")
    dqa_d = nc.dram_tensor("dq_a", [128, 9 * 128], bf, kind="ExternalInput")
    kvw_d = nc.dram_tensor("kvw", [DIM, 2 * DIM], bf, kind="ExternalInput")
    projw_d = nc.dram_tensor("projw", [DIM, DIM], bf, kind="ExternalInput")
    ident_d = nc.dram_tensor("ident", [128, 128], bf, kind="ExternalInput")
    onesb_d = nc.dram_tensor("onesb", [128, 1], bf, kind="ExternalInput")
    vecs_d = nc.dram_tensor("vecs", [DIM, VC_N], f32, kind="ExternalInput")
    vecspb_d = nc.dram_tensor("vecspb", [128, 8], f32, kind="ExternalInput")
    out_d = nc.dram_tensor("out", [DIM, LCORE], f32, kind="ExternalOutput")

    with tile.TileContext(nc) as tc:
        with tc.tile_pool(name="persist", bufs=1) as pp, \
             tc.tile_pool(name="dram", bufs=1, space="DRAM") as dramp:

            vecs = [pp.tile([128, VC_N], f32, tag="vec_a", name="vec_a"),
                    pp.tile([64, VC_N], f32, tag="vec_b", name="vec_b")]
            nc.scalar.dma_start(vecs[0][:], vecs_d.ap()[0:128, :])
            nc.scalar.dma_start(vecs[1][:], vecs_d.ap()[128:192, :])
            vecspb = pp.tile([128, 8], f32, tag="vec_pb", name="vec_pb")
            nc.scalar.dma_start(vecspb[:], vecspb_d.ap()[:, :])
            ident = pp.tile([128, 128], bf, tag="ident", name="ident")
            nc.scalar.dma_start(ident[:], ident_d.ap()[:, :])
            onesb = pp.tile([128, 1], bf, tag="onesb", name="onesb")
            nc.scalar.dma_start(onesb[:], onesb_d.ap()[:, :])

            fmap2a = pp.tile([128, SR, SC], bf, tag="fm_a", name="fm_a")
            # F2B: partitions 0..64 hold fmap2's channel remainder, 64..128
            # hold kvpre's k channels 128..192 (packed for one dw unit)
            F2B = pp.tile([128, SR, SC], bf, tag="f2b", name="f2b")
            nc.gpsimd.memset(fmap2a[:, 3:37, 3:4], 0.0)
            nc.gpsimd.memset(fmap2a[:, 3:37, 68:69], 0.0)
            nc.gpsimd.memset(F2B[0:64, 3:37, 3:4], 0.0)
            nc.gpsimd.memset(F2B[0:64, 3:37, 68:69], 0.0)

            def mm(ps_ap, w_ap, x_ap, start, stop, skip=False):
                nc.tensor.matmul(ps_ap, w_ap, x_ap, start=start, stop=stop,
                                 skip_group_check=skip)

            def evac_rows(eng, dst, dst_p0, msz, ps, r0, rows, bias_tile,
                          cb, cbmt, cbmb, mask_tile, first, last, tk, bk):
                """psum block -> bf16 slab rows [r0, r0+rows) cols [4,68),
                multiplying the tk/bk boundary rows by the validity mask."""
                psv = ps[:, :].rearrange("p (r c) -> p r c", r=rows)
                segs = []
                if first and tk:
                    segs.append((0, tk, VC_MT, cbmt))
                    segs.append((tk, rows - tk, None, cb))
                elif last and bk:
                    segs.append((0, rows - bk, None, cb))
                    segs.append((rows - bk, bk, VC_MB, cbmb))
                else:
                    segs.append((0, rows, None, cb))
                for (o, n, mcol, bcol) in segs:
                    d = dst[dst_p0:dst_p0 + msz, r0 + o:r0 + o + n, 4:68]
                    s = psv[0:msz, o:o + n, :]
                    bias = bias_tile[:, bcol:bcol + 1]
                    if eng == "s":
                        if mcol is None:
                            nc.scalar.activation(d, s, AF.Identity, bias=bias)
                        else:
                            nc.scalar.activation(
                                d, s, AF.Identity, bias=bias,
                                scale=mask_tile[:, mcol:mcol + 1])
                    else:
                        if mcol is None:
                            nc.vector.tensor_scalar_add(d, s, bias)
                        else:
                            nc.vector.tensor_scalar(
                                d, s, mask_tile[:, mcol:mcol + 1], bias,
                                op0=ALU.mult, op1=ALU.add)

            # ---------------- Phase 1: experts + ex_out ----------------
            with tc.tile_pool(name="ph1", bufs=1) as p1, \
                 tc.tile_pool(name="psA", bufs=8, space="PSUM") as psA:
                xin = [p1.tile([128, SR, SC], bf, tag="x_a", name="x_a"),
                       p1.tile([64, SR, SC], bf, tag="x_b", name="x_b")]
                nc.sync.dma_start(xin[0][:], xin_d.ap()[0:128, :, :])
                diag_w = {}
                for i in dw_solo + [s for (j, _) in pairs
                                    for s in (2 * j, 2 * j + 1)]:
                    kk = KSZ[sel[i]] ** 2
                    diag_w[i] = p1.tile([128, kk, 128], bf, tag=f"gw{i}_a",
                                        name=f"gw{i}_a")
                    nc.sync.dma_start(
                        diag_w[i][:],
                        diag_d[i].ap().rearrange("p (s c) -> p s c", s=kk))
                nc.sync.dma_start(xin[1][:], xin_d.ap()[128:192, :, :])
                xb2 = None
                if pairs:
                    xb2 = p1.tile([128, SR, SC], bf, tag="xb2", name="xb2")
                    nc.sync.dma_start(xb2[:], xb2_d.ap()[:, :, :])
                xb2s = None
                if dense_slots:
                    xb2s = p1.tile([128, SR, SC], bf, tag="xb2s", name="xb2s")
                    nc.scalar.dma_start(xb2s[:], xb2s_d.ap()[:, :, :])
                diagb_w = {}
                for i in dw_solo:
                    kk = KSZ[sel[i]] ** 2
                    diagb_w[i] = p1.tile([64, kk, 64], bf, tag=f"gw{i}_b",
                                         name=f"gw{i}_b")
                    nc.scalar.dma_start(
                        diagb_w[i][:],
                        diagb_d[i].ap().rearrange("p (s c) -> p s c", s=kk))
                pb_w = {}
                for (j, ks) in pairs:
                    kk = ks * ks
                    pb_w[j] = p1.tile([128, kk, 128], bf, tag=f"pbw{j}",
                                      name=f"pbw{j}")
                    nc.sync.dma_start(
                        pb_w[j][:],
                        pbdiag_d[j].ap().rearrange("p (s c) -> p s c", s=kk))
                densea_w = {}
                densep_w = {}
                for i in dense_slots:
                    kk = KSZ[sel[i]] ** 2
                    ns = len(_b_pair_sets(KSZ[sel[i]]))
                    densea_w[i] = p1.tile([128, kk, DIM], bf, tag=f"dwa{i}",
                                          name=f"dwa{i}")
                    nc.scalar.dma_start(
                        densea_w[i][:],
                        densea_d[i].ap().rearrange("p (s c) -> p s c", s=kk))
                    densep_w[i] = p1.tile([128, ns, DIM], bf, tag=f"dwp{i}",
                                          name=f"dwp{i}")
                    nc.scalar.dma_start(
                        densep_w[i][:],
                        densep_d[i].ap().rearrange("p (s c) -> p s c", s=ns))
                wexout = p1.tile([128, 6, 9, DIM], bf, tag="wx", name="wx")
                nc.scalar.dma_start(
                    wexout[:], wexout_d.ap().rearrange(
                        "p (e s c) -> p e s c", e=6, s=9))

                A = [p1.tile([128, SR, SC], bf, tag=f"oa{i}", name=f"oa{i}")
                     for i in range(TOPK)]
                PB = [p1.tile([128, SR, SC], bf, tag=f"pb{j}", name=f"pb{j}")
                      for j in range(2)]
                for t_ in A + PB:
                    nc.gpsimd.memset(t_[:, 2:38, 3:4], 0.0)
                    nc.gpsimd.memset(t_[:, 2:38, 68:69], 0.0)

                # dw1x1 experts first (vector-only; runs during weight DMA)
                for sidx, i in enumerate(dw1_slots):
                    base = VC_DW1 + 6 * sidx
                    for mi, (m0, msz) in enumerate(CH):
                        if mi == 0:
                            dst, dp0 = A[i], 0
                        else:
                            dst, dp0 = PB[i // 2], 64 * (i % 2)
                        for (rr0, rr1, cw, cb) in (
                                (2, 4, base + 2, base + 3),
                                (4, 36, base + 0, base + 1),
                                (36, 38, base + 4, base + 5)):
                            nc.vector.tensor_scalar(
                                dst[dp0:dp0 + msz, rr0:rr1, 4:68],
                                xin[mi][:, rr0:rr1, 4:68],
                                vecs[mi][:, cw:cw + 1],
                                vecs[mi][:, cb:cb + 1],
                                op0=ALU.mult, op1=ALU.add)

                def evac_groups(eng, pst, dst, dp0, msz, bias_tile, cb, cmt,
                                cmb, mask_tile, groups, tk, bk):
                    for gi, (r0, rows) in enumerate(groups):
                        evac_rows(eng, dst, dp0, msz, pst[gi], r0, rows,
                                  bias_tile, cb, cmt, cmb, mask_tile,
                                  gi == 0, gi == len(groups) - 1, tk, bk)

                # dw>1 experts, chunk a (diag 128)
                for i in dw_solo + [s for (j, _) in pairs
                                    for s in (2 * j, 2 * j + 1)]:
                    ks, p = KSZ[sel[i]], PADS[sel[i]]
                    kk = ks * ks
                    pst = [psA.tile([128, rows * 64], f32, tag="pA", name="pA")
                           for (r0, rows) in G_E]
                    for si in range(kk):
                        dy, dx = si // ks, si % ks
                        w_ap = diag_w[i][:, si, :]
                        for gi, (r0, rows) in enumerate(G_E):
                            mm(pst[gi][:, :], w_ap,
                               xin[0][:, r0 + dy - p:r0 + dy - p + rows,
                                      4 + dx - p:4 + dx - p + 64],
                               si == 0, si == kk - 1)
                    evac_groups("s", pst, A[i], 0, 128, vecs[0], VC_BIAS + i,
                                VC_BIAS_MT + i, VC_BIAS_MB + i, vecs[0],
                                G_E, 2, 2)

                # dw>1 experts, chunk b: packed pairs on [b;b]
                for (j, ks) in pairs:
                    p = PADS[sel[2 * j]]
                    kk = ks * ks
                    pst = [psA.tile([128, rows * 64], f32, tag="pA", name="pA")
                           for (r0, rows) in G_E]
                    for si in range(kk):
                        dy, dx = si // ks, si % ks
                        w_ap = pb_w[j][:, si, :]
                        for gi, (r0, rows) in enumerate(G_E):
                            mm(pst[gi][:, :], w_ap,
                               xb2[:, r0 + dy - p:r0 + dy - p + rows,
                                   4 + dx - p:4 + dx - p + 64],
                               si == 0, si == kk - 1)
                    evac_groups("v", pst, PB[j], 0, 128, vecspb, 3 * j,
                                3 * j + 1, 3 * j + 2, vecspb, G_E, 2, 2)

                # dw>1 experts, chunk b: unpaired fallback
                for i in dw_solo:
                    ks, p = KSZ[sel[i]], PADS[sel[i]]
                    kk = ks * ks
                    pst = [psA.tile([64, rows * 64], f32, tag="pA", name="pA")
                           for (r0, rows) in G_E]
                    for si in range(kk):
                        dy, dx = si // ks, si % ks
                        w_ap = diagb_w[i][:, si, :]
                        for gi, (r0, rows) in enumerate(G_E):
                            mm(pst[gi][:, :], w_ap,
                               xin[1][:, r0 + dy - p:r0 + dy - p + rows,
                                      4 + dx - p:4 + dx - p + 64],
                               si == 0, si == kk - 1)
                    jj = i // 2
                    evac_groups("v", pst, PB[jj], 64 * (i % 2), 64,
                                vecspb[64 * (i % 2):64 * (i % 2) + 64, :],
                                3 * jj, 3 * jj + 1, 3 * jj + 2,
                                vecspb[64 * (i % 2):64 * (i % 2) + 64, :],
                                G_E, 2, 2)

                # dense experts: chunk-a shifts + paired chunk-b on [b;b<<1]
                for i in dense_slots:
                    ks, p = KSZ[sel[i]], PADS[sel[i]]
                    kk = ks * ks
                    bsets = _b_pair_sets(ks)
                    for mi, (m0, msz) in enumerate(CH):
                        pst = [psA.tile([128, rows * 64], f32, tag="pA",
                                        name="pA")
                               for (r0, rows) in G_E]
                        nmm = kk + len(bsets)
                        c = 0
                        for si in range(kk):
                            dy, dx = si // ks, si % ks
                            w_ap = densea_w[i][:, si, m0:m0 + msz]
                            for gi, (r0, rows) in enumerate(G_E):
                                mm(pst[gi][:msz, :], w_ap,
                                   xin[0][:, r0 + dy - p:r0 + dy - p + rows,
                                          4 + dx - p:4 + dx - p + 64],
                                   c == 0, c == nmm - 1)
                            c += 1
                        for bi, (dy, dx, _) in enumerate(bsets):
                            w_ap = densep_w[i][:, bi, m0:m0 + msz]
                            for gi, (r0, rows) in enumerate(G_E):
                                mm(pst[gi][:msz, :], w_ap,
                                   xb2s[:, r0 + dy - p:r0 + dy - p + rows,
                                        4 + dx - p:4 + dx - p + 64],
                                   c == 0, c == nmm - 1)
                            c += 1
                        if mi == 0:
                            evac_groups("s", pst, A[i], 0, 128, vecs[0],
                                        VC_BIAS + i, VC_BIAS_MT + i,
                                        VC_BIAS_MB + i, vecs[0], G_E, 2, 2)
                        else:
                            jj = i // 2
                            o = 64 * (i % 2)
                            evac_groups("v", pst, PB[jj], o, 64,
                                        vecspb[o:o + 64, :], 3 * jj,
                                        3 * jj + 1, 3 * jj + 2,
                                        vecspb[o:o + 64, :], G_E, 2, 2)

                # ex_out: fmap2 = 3x3 conv over the 6 packed 128-chunks
                OTC = [A[0], A[1], A[2], A[3], PB[0], PB[1]]
                for mi, (m0, msz) in enumerate(CH):
                    pst = [psA.tile([128, rows * 64], f32, tag="pA", name="pA")
                           for (r0, rows) in G_F]
                    nmm = 6 * 9
                    c = 0
                    for cc in range(6):
                        for si in range(9):
                            dy, dx = si // 3, si % 3
                            w_ap = wexout[:, cc, si, m0:m0 + msz]
                            for gi, (r0, rows) in enumerate(G_F):
                                mm(pst[gi][:msz, :], w_ap,
                                   OTC[cc][:, r0 + dy - 1:r0 + dy - 1 + rows,
                                           3 + dx:3 + dx + 64],
                                   c == 0, c == nmm - 1)
                            c += 1
                    if mi == 0:
                        evac_groups("s", pst, fmap2a, 0, 128, vecs[0], VC_XB,
                                    VC_XB_MT, VC_XB_MB, vecs[0], G_F, 1, 1)
                    else:
                        evac_groups("v", pst, F2B, 0, 64, vecs[1], VC_XB,
                                    VC_XB_MT, VC_XB_MB, vecs[1], G_F, 1, 1)

            # ---------------- Phase 2: attention ----------------
            with tc.tile_pool(name="ph2", bufs=1) as p2, \
                 tc.tile_pool(name="psB", bufs=4, space="PSUM") as psB, \
                 tc.tile_pool(name="psW", bufs=4, space="PSUM") as psW:
                kvw = [p2.tile([128, 2 * DIM], bf, tag="kvw_a", name="kvw_a"),
                       p2.tile([64, 2 * DIM], bf, tag="kvw_b", name="kvw_b")]
                nc.sync.dma_start(kvw[0][:], kvw_d.ap()[0:128, :])
                nc.sync.dma_start(kvw[1][:], kvw_d.ap()[128:192, :])
                projw = [p2.tile([128, DIM], bf, tag="pw_a", name="pw_a"),
                         p2.tile([64, DIM], bf, tag="pw_b", name="pw_b")]
                nc.sync.dma_start(projw[0][:], projw_d.ap()[0:128, :])
                nc.sync.dma_start(projw[1][:], projw_d.ap()[128:192, :])
                dqa = p2.tile([128, 9, 128], bf, tag="dqa", name="dqa")
                nc.sync.dma_start(dqa[:], dqa_d.ap().rearrange("p (s c) -> p s c", s=9))
                dqkb = p2.tile([128, 9, 128], bf, tag="dqkb", name="dqkb")
                nc.sync.dma_start(dqkb[:], dqkb_d.ap().rearrange("p (s c) -> p s c", s=9))
                dkv0 = p2.tile([128, 9, 128], bf, tag="dkv0", name="dkv0")
                nc.sync.dma_start(dkv0[:], dkv0_d.ap().rearrange("p (s c) -> p s c", s=9))
                dkv2 = p2.tile([128, 9, 128], bf, tag="dkv2", name="dkv2")
                nc.sync.dma_start(dkv2[:], dkv2_d.ap().rearrange("p (s c) -> p s c", s=9))
                dkv1v = p2.tile([64, 9, 64], bf, tag="dkv1v", name="dkv1v")
                nc.sync.dma_start(dkv1v[:], dkv1v_d.ap().rearrange("p (s c) -> p s c", s=9))

                kp0 = p2.tile([128, SR, SC], bf, tag="kp0", name="kp0")
                kp1v = p2.tile([64, SR, SC], bf, tag="kp1v", name="kp1v")
                kp2 = p2.tile([128, SR, SC], bf, tag="kp2", name="kp2")

                def kvpre_m(m, dsts, groups=G_K):
                    """1x1(fmap2) for kv channel chunk m; dsts: list of
                    (psum_p0, size, dst_tile, dst_p0)."""
                    for gi, (r0, rc) in enumerate(groups):
                        ps = psB.tile([128, rc * 66], f32, tag="b", name="psb")
                        for ki in range(2):
                            src = fmap2a if ki == 0 else F2B
                            mm(ps[:, :], kvw[ki][:, 128 * m:128 * (m + 1)],
                               src[0:(128 if ki == 0 else 64), r0:r0 + rc, 3:69],
                               ki == 0, ki == 1)
                        psv = ps[:, :].rearrange("p (r c) -> p r c", r=rc)
                        for ei, (p0, sz, dt_, d0) in enumerate(dsts):
                            d = dt_[d0:d0 + sz, r0:r0 + rc, 3:69]
                            s = psv[p0:p0 + sz, :, :]
                            if (gi + ei) % 2 == 0:
                                nc.scalar.copy(d, s)
                            else:
                                nc.vector.tensor_copy(d, s)

                kvpre_m(0, [(0, 128, kp0, 0)])
                kvpre_m(1, [(0, 64, F2B, 64), (64, 64, kp1v, 0)])

                q_sb = [p2.tile([128, LCORE], bf, tag="q_a", name="q_a"),
                        p2.tile([64, LCORE], bf, tag="q_b", name="q_b")]
                k_sb = [p2.tile([128, LCORE], bf, tag="k_a", name="k_a"),
                        p2.tile([64, LCORE], bf, tag="k_b", name="k_b")]
                v128 = p2.tile([128, LCORE], bf, tag="v128", name="v128")
                v64 = p2.tile([64, LCORE], bf, tag="v64", name="v64")

                ecnt = 0

                def dw_unit(diag, src, srcn, writes):
                    # weight-stationary: each shift's diag streams all four
                    # row-groups back-to-back so LDWEIGHTS amortizes
                    nonlocal ecnt
                    pst = [psW.tile([128, 512], f32, tag="w", name="psw")
                           for _ in range(4)]
                    for si in range(9):
                        dy, dx = si // 3, si % 3
                        w_ap = diag[:, si, :]
                        for t in range(4):
                            r0 = 4 + 8 * t + dy - 1
                            mm(pst[t][0:srcn, :], w_ap,
                               src[:, r0:r0 + 8, 3 + dx:3 + dx + 64],
                               si == 0, si == 8)
                    for t in range(4):
                        for (p0, sz, dst, d0) in writes:
                            d = dst[d0:d0 + sz, 512 * t:512 * (t + 1)]
                            s = pst[t][p0:p0 + sz, :]
                            if ecnt % 2 == 0:
                                nc.scalar.copy(d, s)
                            else:
                                nc.vector.tensor_copy(d, s)
                            ecnt += 1

                def transpose_t(src, dst, t):
                    pt = psW.tile([128, 128], bf, tag="w", name="pt")
                    nc.tensor.transpose(
                        pt[:, 0:128], src[0][:, 128 * t:128 * (t + 1)],
                        ident[:])
                    pt2 = psW.tile([128, 128], bf, tag="w", name="pt2")
                    nc.tensor.transpose(
                        pt2[:, 0:64], src[1][:, 128 * t:128 * (t + 1)],
                        ident[0:64, 0:64])
                    nc.vector.tensor_copy(dst[:, t, 0:128], pt[:, 0:128])
                    if dst is qT:
                        nc.vector.tensor_copy(dst[:, t, 128:192], pt2[:, 0:64])
                    else:
                        nc.scalar.copy(dst[:, t, 128:192], pt2[:, 0:64])

                qT = p2.tile([128, 16, DIM], bf, tag="qT", name="qT")
                kT = p2.tile([128, 16, DIM], bf, tag="kT", name="kT")

                # interleave psB-based kvpre m2 between psW units so PE keeps
                # streaming across unit-boundary PSUM recycling waits
                dw_unit(dqa, fmap2a, 128, [(0, 128, q_sb[0], 0)])
                kvpre_m(2, [(0, 128, kp2, 0)], groups=G_K[0:3])
                dw_unit(dqkb, F2B, 128,
                        [(0, 64, q_sb[1], 0), (64, 64, k_sb[1], 0)])
                kvpre_m(2, [(0, 128, kp2, 0)], groups=G_K[3:5])

                sq = p2.tile([128, LCORE], bf, tag="sq", name="sq")
                qss = [p2.tile([128, 1], f32, tag="qss_a", name="qss_a"),
                       p2.tile([64, 1], f32, tag="qss_b", name="qss_b")]
                kss = [p2.tile([128, 1], f32, tag="kss_a", name="kss_a"),
                       p2.tile([64, 1], f32, tag="kss_b", name="kss_b")]
                for ci in range(2):
                    n = 128 if ci == 0 else 64
                    nc.scalar.activation(sq[:n, :], q_sb[ci][:], AF.Square,
                                         accum_out=qss[ci][:])

                for t in range(16):
                    transpose_t(q_sb, qT, t)
                dw_unit(dkv0, kp0, 128, [(0, 128, k_sb[0], 0)])
                for ci in range(2):
                    n = 128 if ci == 0 else 64
                    nc.scalar.activation(sq[:n, :], k_sb[ci][:], AF.Square,
                                         accum_out=kss[ci][:])

                G2 = psB.tile([128, 384], f32, tag="b", name="G2")
                for t in range(16):
                    transpose_t(k_sb, kT, t)
                    mm(G2[:, 0:192], qT[:, t, 0:128], kT[:, t, 0:192],
                       t == 0, t == 15, skip=True)
                    mm(G2[0:64, 192:384], qT[:, t, 128:192], kT[:, t, 0:192],
                       t == 0, t == 15, skip=True)

                # staging tile for the paired AllReduce
                stag = p2.tile([128, 68], f32, tag="stag", name="stag")
                nc.gpsimd.memset(stag[:], 0.0)
                nc.vector.tensor_copy(stag[:, 0:1], qss[0][:])
                nc.vector.tensor_copy(stag[0:64, 1:2], qss[1][:])
                nc.vector.tensor_copy(stag[:, 2:3], kss[0][:])
                nc.vector.tensor_copy(stag[0:64, 3:4], kss[1][:])
                for h in range(4):
                    nc.vector.tensor_copy(
                        stag[32 * h:32 * h + 32, 4:36],
                        G2[32 * h:32 * h + 32, 32 * h:32 * h + 32])
                for h in range(2):
                    nc.vector.tensor_copy(
                        stag[32 * h:32 * h + 32, 36:68],
                        G2[32 * h:32 * h + 32, 320 + 32 * h:352 + 32 * h])

                arin = dramp.tile([128, 68], f32, tag="arin", name="arin")
                arout = dramp.tile([128, 68], f32, tag="arout", name="arout")
                nc.gpsimd.dma_start(arin[:], stag[:])
                nc.gpsimd.collective_compute(
                    "AllReduce", ALU.add, replica_groups=RG,
                    ins=[arin.opt()], outs=[arout.opt()])
                stagR = p2.tile([128, 68], f32, tag="stagR", name="stagR")
                nc.gpsimd.dma_start(stagR[:], arout[:])

                # v-side work hides under the collective
                dw_unit(dkv1v, kp1v, 64, [(0, 64, v128, 0)])
                dw_unit(dkv2, kp2, 128,
                        [(0, 64, v128, 64), (64, 64, v64, 0)])

                # inverse norms; temperature folded into q scale
                qsc = [p2.tile([128, 1], f32, tag="qsc_a", name="qsc_a"),
                       p2.tile([64, 1], f32, tag="qsc_b", name="qsc_b")]
                ksc = [p2.tile([128, 1], f32, tag="ksc_a", name="ksc_a"),
                       p2.tile([64, 1], f32, tag="ksc_b", name="ksc_b")]
                for ci in range(2):
                    n = 128 if ci == 0 else 64
                    nc.scalar.sqrt(qsc[ci][:], stagR[0:n, 0 + ci:1 + ci])
                    nc.vector.reciprocal(qsc[ci][:], qsc[ci][:])
                    nc.vector.tensor_mul(qsc[ci][:], qsc[ci][:],
                                         vecs[ci][:, VC_TEMP:VC_TEMP + 1])
                    nc.scalar.sqrt(ksc[ci][:], stagR[0:n, 2 + ci:3 + ci])
                    nc.vector.reciprocal(ksc[ci][:], ksc[ci][:])

                # per-head softmax into block-diagonal attn^T tiles.
                # logits = G*qinv*kinv*temp are bounded (|cos| <= temp), so
                # exp is computed without max subtraction.
                bdA = p2.tile([128, 128], bf, tag="bdA", name="bdA")
                bdB = p2.tile([64, 64], bf, tag="bdB", name="bdB")
                nc.gpsimd.memset(bdA[:], 0.0)
                nc.gpsimd.memset(bdB[:], 0.0)
                for h in range(HEADS):
                    ci, hb = (0, h) if h < 4 else (1, h - 4)
                    c0 = 32 * hb
                    if h < 4:
                        blk = stagR[c0:c0 + 32, 4:36]
                    else:
                        blk = stagR[c0:c0 + 32, 36:68]
                    s1 = p2.tile([32, 32], f32, tag="s1", bufs=3, name="s1")
                    nc.vector.tensor_scalar_mul(s1[:], blk, qsc[ci][c0:c0 + 32, :])
                    s2 = p2.tile([32, 32], f32, tag="s2", bufs=3, name="s2")
                    nc.vector.transpose(s2[:], s1[:])
                    nc.vector.tensor_scalar_mul(s2[:], s2[:], ksc[ci][c0:c0 + 32, :])
                    bd = bdA if h < 4 else bdB
                    nc.scalar.activation(bd[c0:c0 + 32, c0:c0 + 32], s2[:], AF.Exp)

                psum_s = psB.tile([128, 1], f32, tag="b", name="pss")
                mm(psum_s[0:128, :], bdA[:, :], onesb[:, :], True, True)
                psum_s2 = psB.tile([64, 1], f32, tag="b", name="pss2")
                mm(psum_s2[0:64, :], bdB[:, :], onesb[0:64, :], True, True)
                sinv = [p2.tile([128, 1], f32, tag="sinv_a", name="sinv_a"),
                        p2.tile([64, 1], f32, tag="sinv_b", name="sinv_b")]
                nc.vector.reciprocal(sinv[0][:], psum_s[:, :])
                nc.vector.reciprocal(sinv[1][:], psum_s2[:, :])

                o_sb = [p2.tile([128, LCORE], bf, tag="osb_a", name="osb_a"),
                        p2.tile([64, LCORE], bf, tag="osb_b", name="osb_b")]
                for t in range(4):
                    po = psB.tile([128, 512], f32, tag="b", name="po")
                    mm(po[:, :], bdA[:, :], v128[:, 512 * t:512 * (t + 1)],
                       True, True)
                    nc.vector.tensor_scalar_mul(
                        o_sb[0][:, 512 * t:512 * (t + 1)], po[:, :], sinv[0][:])
                    po2 = psB.tile([64, 512], f32, tag="b", name="po2")
                    mm(po2[:, :], bdB[:, :], v64[:, 512 * t:512 * (t + 1)],
                       True, True)
                    nc.scalar.activation(
                        o_sb[1][:, 512 * t:512 * (t + 1)], po2[:, :],
                        AF.Copy, scale=sinv[1][:])

                for mi, (m0, msz) in enumerate(CH):
                    for t in range(4):
                        ps = psB.tile([128, 512], f32, tag="b", name="psp")
                        for ki in range(2):
                            mm(ps[:msz, :], projw[ki][:, m0:m0 + msz],
                               o_sb[ki][:, 512 * t:512 * (t + 1)],
                               ki == 0, ki == 1)
                        st = p2.tile([128, 512], f32, tag="fo_st", bufs=3,
                                     name="fo_st")
                        if (mi * 4 + t) % 2 == 0:
                            nc.scalar.copy(st[:msz, :], ps[:msz, :])
                        else:
                            nc.vector.tensor_copy(st[:msz, :], ps[:msz, :])
                        nc.sync.dma_start(
                            out_d.ap()[m0:m0 + msz, 512 * t:512 * (t + 1)],
                            st[:msz, :])

    nc.compile()
    import os
    trace = bool(os.environ.get("KERNEL_TRACE"))
    res = run_bass_kernel_spmd(nc, host_inputs, core_ids=list(range(NCORES)),
                               trace=trace)
    global LAST_EXEC_NS
    LAST_EXEC_NS = res.exec_time_ns
    return res


LAST_EXEC_NS = None


def _prep_inputs(sel, inputs):
    """Build per-core in_maps (weights shared; xin slab + vecs per core)."""
    I = np.asarray(inputs["I"], dtype=np.float32)
    ex_ws = [np.asarray(inputs[f"ex_w{j}"], dtype=np.float32) for j in range(12)]
    ex_bs = [np.asarray(inputs[f"ex_b{j}"], dtype=np.float32) for j in range(12)]

    dense_slots = [i for i in range(TOPK) if GROUPS[sel[i]] == 1]
    dw1_slots = [i for i in range(TOPK) if GROUPS[sel[i]] == DIM and KSZ[sel[i]] == 1]
    pairs = _dw_pairs(sel)
    paired = {2 * j for (j, _) in pairs} | {2 * j + 1 for (j, _) in pairs}
    dw_solo = [i for i in range(TOPK)
               if GROUPS[sel[i]] == DIM and KSZ[sel[i]] > 1 and i not in paired]

    shared = {}
    for i in dense_slots:
        j = sel[i]
        ks = KSZ[j]
        kk = ks * ks
        w = ex_ws[j]  # [out, in, k, k]
        wa = w[:, 0:128].transpose(1, 2, 3, 0).reshape(128, kk, DIM)
        shared[f"e{i}_wa"] = np.ascontiguousarray(
            wa.reshape(128, kk * DIM)).astype(BF16)
        bsets = _b_pair_sets(ks)
        wp = np.zeros((128, len(bsets), DIM), dtype=np.float32)
        for bi, (dy, dx, dx2) in enumerate(bsets):
            wp[0:64, bi] = w[:, 128:192, dy, dx].T
            if dx2 is not None:
                wp[64:128, bi] = w[:, 128:192, dy, dx2].T
        shared[f"e{i}_wp"] = wp.reshape(128, len(bsets) * DIM).astype(BF16)

    for i in dw_solo + sorted(paired):
        j = sel[i]
        ks = KSZ[j]
        kk = ks * ks
        wv = ex_ws[j][:, 0, :, :].reshape(DIM, kk)
        da = np.zeros((128, kk, 128), dtype=np.float32)
        for c in range(128):
            da[c, :, c] = wv[c]
        shared[f"e{i}_da"] = da.reshape(128, kk * 128).astype(BF16)
        if i in dw_solo:
            db = np.zeros((64, kk, 64), dtype=np.float32)
            for c in range(64):
                db[c, :, c] = wv[128 + c]
            shared[f"e{i}_db"] = db.reshape(64, kk * 64).astype(BF16)
    for (jj, ks) in pairs:
        kk = ks * ks
        w0 = ex_ws[sel[2 * jj]][:, 0, :, :].reshape(DIM, kk)
        w1 = ex_ws[sel[2 * jj + 1]][:, 0, :, :].reshape(DIM, kk)
        dpb = np.zeros((128, kk, 128), dtype=np.float32)
        for c in range(64):
            dpb[c, :, c] = w0[128 + c]
            dpb[64 + c, :, 64 + c] = w1[128 + c]
        shared[f"e_pb{jj}"] = dpb.reshape(128, kk * 128).astype(BF16)

    exw = np.asarray(inputs["ex_out_w"], dtype=np.float32)  # [192, 768, 3, 3]
    wx = np.zeros((128, 6, 9, DIM), dtype=np.float32)
    for cc in range(4):
        blk = exw[:, 192 * cc:192 * cc + 128, :, :]
        wx[:, cc, :, :] = blk.transpose(1, 2, 3, 0).reshape(128, 9, DIM)
    for half in range(2):
        s0, s1 = 2 * half, 2 * half + 1
        b0 = exw[:, 192 * s0 + 128:192 * (s0 + 1), :, :]
        b1 = exw[:, 192 * s1 + 128:192 * (s1 + 1), :, :]
        wx[0:64, 4 + half] = b0.transpose(1, 2, 3, 0).reshape(64, 9, DIM)
        wx[64:128, 4 + half] = b1.transpose(1, 2, 3, 0).reshape(64, 9, DIM)
    shared["wexout"] = wx.reshape(128, 6 * 9 * DIM).astype(BF16)

    kvdw = np.asarray(inputs["kv_dw_w"], dtype=np.float32)[:, 0].reshape(384, 9)
    qdw = np.asarray(inputs["q_dw_w"], dtype=np.float32)[:, 0].reshape(DIM, 9)
    d0 = np.zeros((128, 9, 128), dtype=np.float32)
    d2 = np.zeros((128, 9, 128), dtype=np.float32)
    dqk = np.zeros((128, 9, 128), dtype=np.float32)
    d1v = np.zeros((64, 9, 64), dtype=np.float32)
    for c in range(128):
        d0[c, :, c] = kvdw[c]          # k channels 0..128
        d2[c, :, c] = kvdw[256 + c]    # v channels 64..192
    for c in range(64):
        dqk[c, :, c] = qdw[128 + c]            # q channels 128..192
        dqk[64 + c, :, 64 + c] = kvdw[128 + c]  # k channels 128..192
        d1v[c, :, c] = kvdw[192 + c]            # v channels 0..64
    shared["dkv0"] = d0.reshape(128, 9 * 128).astype(BF16)
    shared["dkv2"] = d2.reshape(128, 9 * 128).astype(BF16)
    shared["dqkb"] = dqk.reshape(128, 9 * 128).astype(BF16)
    shared["dkv1v"] = d1v.reshape(64, 9 * 64).astype(BF16)
    da = np.zeros((128, 9, 128), dtype=np.float32)
    for c in range(128):
        da[c, :, c] = qdw[c]
    shared["dq_a"] = da.reshape(128, 9 * 128).astype(BF16)

    kvw = np.asarray(inputs["kv_w"], dtype=np.float32)[:, :, 0, 0]
    shared["kvw"] = np.ascontiguousarray(kvw.T).astype(BF16)
    pw = np.asarray(inputs["proj_w"], dtype=np.float32)[:, :, 0, 0]
    shared["projw"] = np.ascontiguousarray(pw.T).astype(BF16)
    shared["ident"] = np.eye(128, dtype=np.float32).astype(BF16)
    shared["onesb"] = np.ones((128, 1), dtype=np.float32).astype(BF16)

    temp = np.asarray(inputs["temperature"], dtype=np.float32).reshape(HEADS)
    exob = np.asarray(inputs["ex_out_b"], dtype=np.float32)

    def build_vecs(mt, mb):
        v = np.zeros((DIM, VC_N), dtype=np.float32)
        for i in range(TOPK):
            if i in dw1_slots:
                continue
            v[:, VC_BIAS + i] = ex_bs[sel[i]]
            v[:, VC_BIAS_MT + i] = ex_bs[sel[i]] * mt
            v[:, VC_BIAS_MB + i] = ex_bs[sel[i]] * mb
        v[:, VC_XB] = exob
        v[:, VC_XB_MT] = exob * mt
        v[:, VC_XB_MB] = exob * mb
        v[:, VC_TEMP] = np.repeat(temp, DIM // HEADS)
        v[:, VC_MT] = mt
        v[:, VC_MB] = mb
        for sidx, i in enumerate(dw1_slots):
            base = VC_DW1 + 6 * sidx
            w = ex_ws[sel[i]][:, 0, 0, 0]
            b = ex_bs[sel[i]]
            v[:, base + 0] = w
            v[:, base + 1] = b
            v[:, base + 2] = w * mt
            v[:, base + 3] = b * mt
            v[:, base + 4] = w * mb
            v[:, base + 5] = b * mb
        return v

    def build_vecspb(mt, mb):
        v = np.zeros((128, 8), dtype=np.float32)
        v[:, VC_MT] = mt
        v[:, VC_MB] = mb
        for jj in range(2):
            for half in range(2):
                i = 2 * jj + half
                if i in dw1_slots:
                    continue  # dw1x1 path adds its own bias
                bb = ex_bs[sel[i]][128:192]
                r = slice(64 * half, 64 * half + 64)
                v[r, 3 * jj + 0] = bb
                v[r, 3 * jj + 1] = bb * mt
                v[r, 3 * jj + 2] = bb * mb
        return v

    vecs_h = [build_vecs(0.0, 1.0), build_vecs(1.0, 0.0)]
    vecspb_h = [build_vecspb(0.0, 1.0), build_vecspb(1.0, 0.0)]

    padded = np.zeros((B, DIM, 72, 72), dtype=np.float32)
    padded[:, :, 4:68, 4:68] = I
    in_maps = []
    for b in range(B):
        for h in range(2):
            m = dict(shared)
            slab = padded[b][:, 32 * h:32 * h + SR, :]
            m["xin"] = np.ascontiguousarray(slab).astype(BF16)
            if dense_slots:
                xb2s = np.zeros((128, SR, SC), dtype=np.float32)
                xb2s[0:64] = slab[128:192]
                xb2s[64:128, :, 0:SC - 1] = slab[128:192, :, 1:SC]
                m["xb2s"] = xb2s.astype(BF16)
            if pairs:
                xb2 = np.concatenate([slab[128:192], slab[128:192]], axis=0)
                m["xb2"] = np.ascontiguousarray(xb2).astype(BF16)
            m["vecs"] = vecs_h[h]
            m["vecspb"] = vecspb_h[h]
            in_maps.append(m)
    return in_maps


def kernel(**inputs) -> np.ndarray:
    I = np.asarray(inputs["I"], dtype=np.float32)
    T = np.asarray(inputs["T"], dtype=np.float32)
    pw = np.asarray(inputs["ca1_proj_w"], dtype=np.float32)
    sel = _select_experts(I, T, pw)
    in_maps = _prep_inputs(sel, inputs)
    res = _build_and_run(sel, in_maps)
    out = np.zeros((B, DIM, H, W), dtype=np.float32)
    for b in range(B):
        for h in range(2):
            out[b, :, 32 * h:32 * h + 32, :] = np.asarray(
                res.results[2 * b + h]["out"],
                dtype=np.float32).reshape(DIM, 32, 64)
    return out


# revision 14
# speedup vs baseline: 1.0650x; 1.0650x over previous
"""Trainium2 Bass kernel for nn_Attention_C_12111807775306.

Structure exploited: attention_ca's output feeds ONLY the top-k expert
selection (batch element 0 alone); the expert conv branches apply to the
original input I. The first channel-attention runs on the host (cheap numpy,
batch 0 only) to pick the 4 experts; the device kernel computes: 4 expert
convs on I -> concat -> 3x3 conv (768->192) -> kv/q convs -> channel
attention -> 1x1 proj.

Sharding: 8 cores = 4 batches x 2 row-halves. Each core holds a 40-row slab
of its batch's (zero-padded) input and computes its 32 output rows plus halo
rows of the intermediate feature maps; the only cross-core communication is
one 26KB paired AllReduce carrying the q/k norm partial sums and the
per-head Gram partial sums. The v-convs are emitted after the AllReduce
trigger so the collective latency hides under PE work.

The 192-channel contraction splits into a 128-chunk and a 64-chunk; 64-chunk
matmuls cost the same as 128 ones, so the host ships duplicated copies of
the channel remainder ([b;b] and [b;b shifted one col]) letting two conv
shifts or two depthwise experts share one full 128-contraction matmul.
Matmul ordering is weight-stationary across row-groups so LDWEIGHTS
amortizes. Compute dtype bf16 (fp32 PSUM), fp32 output.
"""
import sys
sys.path.insert(0, "/opt/trn_rl_repo")
import numpy as np
import ml_dtypes

DIM = 192
HEADS = 6
B = 4
H = 64
W = 64
L = H * W
TOPK = 4
PADS = [0, 1, 2] * 4
KSZ = [1, 3, 5] * 4
GROUPS = [1] * 6 + [DIM] * 6
BF16 = ml_dtypes.bfloat16

SR = 40            # slab rows per core (4 pad + 32 out + 4 halo)
SC = 72            # slab cols (4 + 64 + 4)
LCORE = 32 * 64    # per-core attention length

G_E = [(2, 8), (10, 8), (18, 8), (26, 8), (34, 4)]   # expert outs rows [2,38)
G_F = [(3, 8), (11, 8), (19, 8), (27, 8), (35, 2)]   # fmap2 rows [3,37)
G_K = [(3, 7), (10, 7), (17, 7), (24, 7), (31, 6)]   # kvpre rows [3,37)

CH = [(0, 128), (128, 64)]

# vecs columns
VC_BIAS = 0        # ..3: expert slot biases (channels; used for A-chunk evac)
VC_XB = 4          # ex_out bias
VC_TEMP = 5
VC_MT = 6          # top mask (0 when rows 2..4 are outside the image)
VC_MB = 7          # bottom mask
VC_BIAS_MT = 8     # ..11
VC_BIAS_MB = 12    # ..15
VC_XB_MT = 16
VC_XB_MB = 17
VC_DW1 = 18        # 6 cols per dw1x1 slot (w, b, w*mt, b*mt, w*mb, b*mb)
VC_N = 32


def _l2n(x):
    return x / np.maximum(np.linalg.norm(x, axis=-1, keepdims=True), 1e-12)


def _select_experts(I, T, ca1_proj_w):
    """Replicate attention_ca + binning for batch 0 only; return top-4 idx."""
    b0I = I[0].astype(np.float64)
    b0T = T[0].astype(np.float64)
    pooled = b0T.reshape(DIM // 4, 4, L).mean(1)
    q = _l2n(b0I.reshape(HEADS, DIM // HEADS, L))
    k = _l2n(pooled.reshape(HEADS, 8, L))
    kt = np.tile(k, (1, 4, 1))
    s = np.einsum("hcl,hdl->hcd", q, kt)
    s = s - s.max(-1, keepdims=True)
    e = np.exp(s)
    attn = e / e.sum(-1, keepdims=True)
    out = np.einsum("hcd,hdl->hcl", attn, kt).reshape(DIM, H, W)
    fmap0 = np.einsum("oi,ihw->ohw", ca1_proj_w[:, :, 0, 0].astype(np.float64), out)
    m = fmap0.mean(axis=(0, 1))
    bins = np.array([m[(i * W) // 12: -(-((i + 1) * W) // 12)].mean()
                     for i in range(12)])
    return [int(x) for x in np.argsort(-bins, kind="stable")[:TOPK]]


def _b_pair_sets(ks):
    """Pairs of dx shifts sharing one 128-contraction matmul on [b; b<<1]."""
    sets = []
    for dy in range(ks):
        dx = 0
        while dx < ks:
            if dx + 1 < ks:
                sets.append((dy, dx, dx + 1))
                dx += 2
            else:
                sets.append((dy, dx, None))
                dx += 1
    return sets


def _dw_pairs(sel):
    """(j, ks) for packable dw pairs (slots 2j, 2j+1 both dw>1, same ksz)."""
    out = []
    for j in range(2):
        s0, s1 = 2 * j, 2 * j + 1
        if (GROUPS[sel[s0]] == DIM and KSZ[sel[s0]] > 1
                and GROUPS[sel[s1]] == DIM and KSZ[sel[s1]] > 1
                and KSZ[sel[s0]] == KSZ[sel[s1]]):
            out.append((j, KSZ[sel[s0]]))
    return out


def _build_and_run(sel, host_inputs):
    import concourse.mybir as mybir
    import concourse.tile as tile
    from concourse import bacc
    from concourse.bass_utils import run_bass_kernel_spmd

    bf = mybir.dt.bfloat16
    f32 = mybir.dt.float32
    AF = mybir.ActivationFunctionType
    ALU = mybir.AluOpType
    NCORES = 8
    RG = [[0, 1], [2, 3], [4, 5], [6, 7]]

    dense_slots = [i for i in range(TOPK) if GROUPS[sel[i]] == 1]
    dw1_slots = [i for i in range(TOPK) if GROUPS[sel[i]] == DIM and KSZ[sel[i]] == 1]
    pairs = _dw_pairs(sel)
    paired = {2 * j for (j, _) in pairs} | {2 * j + 1 for (j, _) in pairs}
    dw_solo = [i for i in range(TOPK)
               if GROUPS[sel[i]] == DIM and KSZ[sel[i]] > 1 and i not in paired]

    nc = bacc.Bacc("TRN2", target_bir_lowering=False, debug=False,
                   enable_asserts=False, num_devices=NCORES)

    # ---- DRAM I/O ----
    xin_d = nc.dram_tensor("xin", [DIM, SR, SC], bf, kind="ExternalInput")
    xb2s_d = (nc.dram_tensor("xb2s", [128, SR, SC], bf, kind="ExternalInput")
              if dense_slots else None)
    xb2_d = (nc.dram_tensor("xb2", [128, SR, SC], bf, kind="ExternalInput")
             if pairs else None)
    densea_d = {}
    densep_d = {}
    for i in dense_slots:
        kk = KSZ[sel[i]] ** 2
        ns = len(_b_pair_sets(KSZ[sel[i]]))
        densea_d[i] = nc.dram_tensor(f"e{i}_wa", [128, kk * DIM], bf,
                                     kind="ExternalInput")
        densep_d[i] = nc.dram_tensor(f"e{i}_wp", [128, ns * DIM], bf,
                                     kind="ExternalInput")
    diag_d = {}
    for i in dw_solo + [2 * j for (j, _) in pairs] + [2 * j + 1 for (j, _) in pairs]:
        kk = KSZ[sel[i]] ** 2
        diag_d[i] = nc.dram_tensor(f"e{i}_da", [128, kk * 128], bf,
                                   kind="ExternalInput")
    diagb_d = {i: nc.dram_tensor(f"e{i}_db", [64, KSZ[sel[i]] ** 2 * 64], bf,
                                 kind="ExternalInput") for i in dw_solo}
    pbdiag_d = {j: nc.dram_tensor(f"e_pb{j}", [128, ks * ks * 128], bf,
                                  kind="ExternalInput") for (j, ks) in pairs}
    wexout_d = nc.dram_tensor("wexout", [128, 6 * 9 * DIM], bf,
                              kind="ExternalInput")
    dkv0_d = nc.dram_tensor("dkv0", [128, 9 * 128], bf, kind="ExternalInput")
    dkv2_d = nc.dram_tensor("dkv2", [128, 9 * 128], bf, kind="ExternalInput")
    dkv1v_d = nc.dram_tensor("dkv1v", [64, 9 * 64], bf, kind="ExternalInput")
    dqkb_d = nc.dram_tensor("dqkb", [128, 9 * 128], bf, kind="ExternalInput")
    dqa_d = nc.dram_tensor("dq_a", [128, 9 * 128], bf, kind="ExternalInput")
    kvw_d = nc.dram_tensor("kvw", [DIM, 2 * DIM], bf, kind="ExternalInput")
    projw_d = nc.dram_tensor("projw", [DIM, DIM], bf, kind="ExternalInput")
    ident_d = nc.dram_tensor("ident", [128, 128], bf, kind="ExternalInput")
    onesb_d = nc.dram_tensor("onesb", [128, 1], bf, kind="ExternalInput")
    vecs_d = nc.dram_tensor("vecs", [DIM, VC_N], f32, kind="ExternalInput")
    vecspb_d = nc.dram_tensor("vecspb", [128, 8], f32, kind="ExternalInput")
    out_d = nc.dram_tensor("out", [DIM, LCORE], f32, kind="ExternalOutput")

    with tile.TileContext(nc) as tc:
        with tc.tile_pool(name="persist", bufs=1) as pp, \
             tc.tile_pool(name="dram", bufs=1, space="DRAM") as dramp:

            vecs = [pp.tile([128, VC_N], f32, tag="vec_a", name="vec_a"),
                    pp.tile([64, VC_N], f32, tag="vec_b", name="vec_b")]
            nc.scalar.dma_start(vecs[0][:], vecs_d.ap()[0:128, :])
            nc.scalar.dma_start(vecs[1][:], vecs_d.ap()[128:192, :])
            vecspb = pp.tile([128, 8], f32, tag="vec_pb", name="vec_pb")
            nc.scalar.dma_start(vecspb[:], vecspb_d.ap()[:, :])
            ident = pp.tile([128, 128], bf, tag="ident", name="ident")
            nc.scalar.dma_start(ident[:], ident_d.ap()[:, :])
            onesb = pp.tile([128, 1], bf, tag="onesb", name="onesb")
            nc.scalar.dma_start(onesb[:], onesb_d.ap()[:, :])

            fmap2a = pp.tile([128, SR, SC], bf, tag="fm_a", name="fm_a")
            # F2B: partitions 0..64 hold fmap2's channel remainder, 64..128
            # hold kvpre's k channels 128..192 (packed for one dw unit)
            F2B = pp.tile([128, SR, SC], bf, tag="f2b", name="f2b")
            nc.gpsimd.memset(fmap2a[:, 3:37, 3:4], 0.0)
            nc.gpsimd.memset(fmap2a[:, 3:37, 68:69], 0.0)
            nc.gpsimd.memset(F2B[0:64, 3:37, 3:4], 0.0)
            nc.gpsimd.memset(F2B[0:64, 3:37, 68:69], 0.0)

            def mm(ps_ap, w_ap, x_ap, start, stop, skip=False):
                nc.tensor.matmul(ps_ap, w_ap, x_ap, start=start, stop=stop,
                                 skip_group_check=skip)

            def evac_rows(eng, dst, dst_p0, msz, ps, r0, rows, bias_tile,
                          cb, cbmt, cbmb, mask_tile, first, last, tk, bk):
                """psum block -> bf16 slab rows [r0, r0+rows) cols [4,68),
                multiplying the tk/bk boundary rows by the validity mask."""
                psv = ps[:, :].rearrange("p (r c) -> p r c", r=rows)
                segs = []
                if first and tk:
                    segs.append((0, tk, VC_MT, cbmt))
                    segs.append((tk, rows - tk, None, cb))
                elif last and bk:
                    segs.append((0, rows - bk, None, cb))
                    segs.append((rows - bk, bk, VC_MB, cbmb))
                else:
                    segs.append((0, rows, None, cb))
                for (o, n, mcol, bcol) in segs:
                    d = dst[dst_p0:dst_p0 + msz, r0 + o:r0 + o + n, 4:68]
                    s = psv[0:msz, o:o + n, :]
                    bias = bias_tile[:, bcol:bcol + 1]
                    if eng == "s":
                        if mcol is None:
                            nc.scalar.activation(d, s, AF.Identity, bias=bias)
                        else:
                            nc.scalar.activation(
                                d, s, AF.Identity, bias=bias,
                                scale=mask_tile[:, mcol:mcol + 1])
                    else:
                        if mcol is None:
                            nc.vector.tensor_scalar_add(d, s, bias)
                        else:
                            nc.vector.tensor_scalar(
                                d, s, mask_tile[:, mcol:mcol + 1], bias,
                                op0=ALU.mult, op1=ALU.add)

            # ---------------- Phase 1: experts + ex_out ----------------
            with tc.tile_pool(name="ph1", bufs=1) as p1, \
                 tc.tile_pool(name="psA", bufs=8, space="PSUM") as psA:
                xin = [p1.tile([128, SR, SC], bf, tag="x_a", name="x_a"),
                       p1.tile([64, SR, SC], bf, tag="x_b", name="x_b")]
                nc.sync.dma_start(xin[0][:], xin_d.ap()[0:128, :, :])
                diag_w = {}
                for i in dw_solo + [s for (j, _) in pairs
                                    for s in (2 * j, 2 * j + 1)]:
                    kk = KSZ[sel[i]] ** 2
                    diag_w[i] = p1.tile([128, kk, 128], bf, tag=f"gw{i}_a",
                                        name=f"gw{i}_a")
                    nc.sync.dma_start(
                        diag_w[i][:],
                        diag_d[i].ap().rearrange("p (s c) -> p s c", s=kk))
                nc.sync.dma_start(xin[1][:], xin_d.ap()[128:192, :, :])
                xb2 = None
                if pairs:
                    xb2 = p1.tile([128, SR, SC], bf, tag="xb2", name="xb2")
                    nc.sync.dma_start(xb2[:], xb2_d.ap()[:, :, :])
                xb2s = None
                if dense_slots:
                    xb2s = p1.tile([128, SR, SC], bf, tag="xb2s", name="xb2s")
                    nc.scalar.dma_start(xb2s[:], xb2s_d.ap()[:, :, :])
                diagb_w = {}
                for i in dw_solo:
                    kk = KSZ[sel[i]] ** 2
                    diagb_w[i] = p1.tile([64, kk, 64], bf, tag=f"gw{i}_b",
                                         name=f"gw{i}_b")
                    nc.scalar.dma_start(
                        diagb_w[i][:],
                        diagb_d[i].ap().rearrange("p (s c) -> p s c", s=kk))
                pb_w = {}
                for (j, ks) in pairs:
                    kk = ks * ks
                    pb_w[j] = p1.tile([128, kk, 128], bf, tag=f"pbw{j}",
                                      name=f"pbw{j}")
                    nc.sync.dma_start(
                        pb_w[j][:],
                        pbdiag_d[j].ap().rearrange("p (s c) -> p s c", s=kk))
                densea_w = {}
                densep_w = {}
                for i in dense_slots:
                    kk = KSZ[sel[i]] ** 2
                    ns = len(_b_pair_sets(KSZ[sel[i]]))
                    densea_w[i] = p1.tile([128, kk, DIM], bf, tag=f"dwa{i}",
                                          name=f"dwa{i}")
                    nc.scalar.dma_start(
                        densea_w[i][:],
                        densea_d[i].ap().rearrange("p (s c) -> p s c", s=kk))
                    densep_w[i] = p1.tile([128, ns, DIM], bf, tag=f"dwp{i}",
                                          name=f"dwp{i}")
                    nc.scalar.dma_start(
                        densep_w[i][:],
                        densep_d[i].ap().rearrange("p (s c) -> p s c", s=ns))
                wexout = p1.tile([128, 6, 9, DIM], bf, tag="wx", name="wx")
                nc.scalar.dma_start(
                    wexout[:], wexout_d.ap().rearrange(
                        "p (e s c) -> p e s c", e=6, s=9))

                A = [p1.tile([128, SR, SC], bf, tag=f"oa{i}", name=f"oa{i}")
                     for i in range(TOPK)]
                PB = [p1.tile([128, SR, SC], bf, tag=f"pb{j}", name=f"pb{j}")
                      for j in range(2)]
                for t_ in A + PB:
                    nc.gpsimd.memset(t_[:, 2:38, 3:4], 0.0)
                    nc.gpsimd.memset(t_[:, 2:38, 68:69], 0.0)

                # warmup AllReduce: syncs the pair and warms the collective
                # path early, hidden under phase-1 compute, so the real
                # AllReduce later doesn't absorb cross-core start skew
                warm_in = dramp.tile([128, 1], f32, tag="win", name="warm_in")
                warm_out = dramp.tile([128, 1], f32, tag="wout",
                                      name="warm_out")
                nc.gpsimd.dma_start(warm_in[:], vecs[0][:, 0:1])
                nc.gpsimd.collective_compute(
                    "AllReduce", ALU.add, replica_groups=RG,
                    ins=[warm_in.opt()], outs=[warm_out.opt()])

                # dw1x1 experts first (vector-only; runs during weight DMA)
                for sidx, i in enumerate(dw1_slots):
                    base = VC_DW1 + 6 * sidx
                    for mi, (m0, msz) in enumerate(CH):
                        if mi == 0:
                            dst, dp0 = A[i], 0
                        else:
                            dst, dp0 = PB[i // 2], 64 * (i % 2)
                        for (rr0, rr1, cw, cb) in (
                                (2, 4, base + 2, base + 3),
                                (4, 36, base + 0, base + 1),
                                (36, 38, base + 4, base + 5)):
                            nc.vector.tensor_scalar(
                                dst[dp0:dp0 + msz, rr0:rr1, 4:68],
                                xin[mi][:, rr0:rr1, 4:68],
                                vecs[mi][:, cw:cw + 1],
                                vecs[mi][:, cb:cb + 1],
                                op0=ALU.mult, op1=ALU.add)

                def evac_groups(eng, pst, dst, dp0, msz, bias_tile, cb, cmt,
                                cmb, mask_tile, groups, tk, bk):
                    for gi, (r0, rows) in enumerate(groups):
                        evac_rows(eng, dst, dp0, msz, pst[gi], r0, rows,
                                  bias_tile, cb, cmt, cmb, mask_tile,
                                  gi == 0, gi == len(groups) - 1, tk, bk)

                # dw>1 experts, chunk a (diag 128)
                for i in dw_solo + [s for (j, _) in pairs
                                    for s in (2 * j, 2 * j + 1)]:
                    ks, p = KSZ[sel[i]], PADS[sel[i]]
                    kk = ks * ks
                    pst = [psA.tile([128, rows * 64], f32, tag="pA", name="pA")
                           for (r0, rows) in G_E]
                    for si in range(kk):
                        dy, dx = si // ks, si % ks
                        w_ap = diag_w[i][:, si, :]
                        for gi, (r0, rows) in enumerate(G_E):
                            mm(pst[gi][:, :], w_ap,
                               xin[0][:, r0 + dy - p:r0 + dy - p + rows,
                                      4 + dx - p:4 + dx - p + 64],
                               si == 0, si == kk - 1)
                    evac_groups("s", pst, A[i], 0, 128, vecs[0], VC_BIAS + i,
                                VC_BIAS_MT + i, VC_BIAS_MB + i, vecs[0],
                                G_E, 2, 2)

                # dw>1 experts, chunk b: packed pairs on [b;b]
                for (j, ks) in pairs:
                    p = PADS[sel[2 * j]]
                    kk = ks * ks
                    pst = [psA.tile([128, rows * 64], f32, tag="pA", name="pA")
                           for (r0, rows) in G_E]
                    for si in range(kk):
                        dy, dx = si // ks, si % ks
                        w_ap = pb_w[j][:, si, :]
                        for gi, (r0, rows) in enumerate(G_E):
                            mm(pst[gi][:, :], w_ap,
                               xb2[:, r0 + dy - p:r0 + dy - p + rows,
                                   4 + dx - p:4 + dx - p + 64],
                               si == 0, si == kk - 1)
                    evac_groups("v", pst, PB[j], 0, 128, vecspb, 3 * j,
                                3 * j + 1, 3 * j + 2, vecspb, G_E, 2, 2)

                # dw>1 experts, chunk b: unpaired fallback
                for i in dw_solo:
                    ks, p = KSZ[sel[i]], PADS[sel[i]]
                    kk = ks * ks
                    pst = [psA.tile([64, rows * 64], f32, tag="pA", name="pA")
                           for (r0, rows) in G_E]
                    for si in range(kk):
                        dy, dx = si // ks, si % ks
                        w_ap = diagb_w[i][:, si, :]
                        for gi, (r0, rows) in enumerate(G_E):
                            mm(pst[gi][:, :], w_ap,
                               xin[1][:, r0 + dy - p:r0 + dy - p + rows,
                                      4 + dx - p:4 + dx - p + 64],
                               si == 0, si == kk - 1)
                    jj = i // 2
                    evac_groups("v", pst, PB[jj], 64 * (i % 2), 64,
                                vecspb[64 * (i % 2):64 * (i % 2) + 64, :],
                                3 * jj, 3 * jj + 1, 3 * jj + 2,
                                vecspb[64 * (i % 2):64 * (i % 2) + 64, :],
                                G_E, 2, 2)

                # dense experts: chunk-a shifts + paired chunk-b on [b;b<<1]
                for i in dense_slots:
                    ks, p = KSZ[sel[i]], PADS[sel[i]]
                    kk = ks * ks
                    bsets = _b_pair_sets(ks)
                    for mi, (m0, msz) in enumerate(CH):
                        pst = [psA.tile([128, rows * 64], f32, tag="pA",
                                        name="pA")
                               for (r0, rows) in G_E]
                        nmm = kk + len(bsets)
                        c = 0
                        for si in range(kk):
                            dy, dx = si // ks, si % ks
                            w_ap = densea_w[i][:, si, m0:m0 + msz]
                            for gi, (r0, rows) in enumerate(G_E):
                                mm(pst[gi][:msz, :], w_ap,
                                   xin[0][:, r0 + dy - p:r0 + dy - p + rows,
                                          4 + dx - p:4 + dx - p + 64],
                                   c == 0, c == nmm - 1)
                            c += 1
                        for bi, (dy, dx, _) in enumerate(bsets):
                            w_ap = densep_w[i][:, bi, m0:m0 + msz]
                            for gi, (r0, rows) in enumerate(G_E):
                                mm(pst[gi][:msz, :], w_ap,
                                   xb2s[:, r0 + dy - p:r0 + dy - p + rows,
                                        4 + dx - p:4 + dx - p + 64],
                                   c == 0, c == nmm - 1)
                            c += 1
                        if mi == 0:
                            evac_groups("s", pst, A[i], 0, 128, vecs[0],
                                        VC_BIAS + i, VC_BIAS_MT + i,
                                        VC_BIAS_MB + i, vecs[0], G_E, 2, 2)
                        else:
                            jj = i // 2
                            o = 64 * (i % 2)
                            evac_groups("v", pst, PB[jj], o, 64,
                                        vecspb[o:o + 64, :], 3 * jj,
                                        3 * jj + 1, 3 * jj + 2,
                                        vecspb[o:o + 64, :], G_E, 2, 2)

                # ex_out: fmap2 = 3x3 conv over the 6 packed 128-chunks
                OTC = [A[0], A[1], A[2], A[3], PB[0], PB[1]]
                for mi, (m0, msz) in enumerate(CH):
                    pst = [psA.tile([128, rows * 64], f32, tag="pA", name="pA")
                           for (r0, rows) in G_F]
                    nmm = 6 * 9
                    c = 0
                    for cc in range(6):
                        for si in range(9):
                            dy, dx = si // 3, si % 3
                            w_ap = wexout[:, cc, si, m0:m0 + msz]
                            for gi, (r0, rows) in enumerate(G_F):
                                mm(pst[gi][:msz, :], w_ap,
                                   OTC[cc][:, r0 + dy - 1:r0 + dy - 1 + rows,
                                           3 + dx:3 + dx + 64],
                                   c == 0, c == nmm - 1)
                            c += 1
                    if mi == 0:
                        evac_groups("s", pst, fmap2a, 0, 128, vecs[0], VC_XB,
                                    VC_XB_MT, VC_XB_MB, vecs[0], G_F, 1, 1)
                    else:
                        evac_groups("v", pst, F2B, 0, 64, vecs[1], VC_XB,
                                    VC_XB_MT, VC_XB_MB, vecs[1], G_F, 1, 1)

            # ---------------- Phase 2: attention ----------------
            with tc.tile_pool(name="ph2", bufs=1) as p2, \
                 tc.tile_pool(name="psB", bufs=4, space="PSUM") as psB, \
                 tc.tile_pool(name="psW", bufs=4, space="PSUM") as psW:
                kvw = [p2.tile([128, 2 * DIM], bf, tag="kvw_a", name="kvw_a"),
                       p2.tile([64, 2 * DIM], bf, tag="kvw_b", name="kvw_b")]
                nc.sync.dma_start(kvw[0][:], kvw_d.ap()[0:128, :])
                nc.sync.dma_start(kvw[1][:], kvw_d.ap()[128:192, :])
                projw = [p2.tile([128, DIM], bf, tag="pw_a", name="pw_a"),
                         p2.tile([64, DIM], bf, tag="pw_b", name="pw_b")]
                nc.sync.dma_start(projw[0][:], projw_d.ap()[0:128, :])
                nc.sync.dma_start(projw[1][:], projw_d.ap()[128:192, :])
                dqa = p2.tile([128, 9, 128], bf, tag="dqa", name="dqa")
                nc.sync.dma_start(dqa[:], dqa_d.ap().rearrange("p (s c) -> p s c", s=9))
                dqkb = p2.tile([128, 9, 128], bf, tag="dqkb", name="dqkb")
                nc.sync.dma_start(dqkb[:], dqkb_d.ap().rearrange("p (s c) -> p s c", s=9))
                dkv0 = p2.tile([128, 9, 128], bf, tag="dkv0", name="dkv0")
                nc.sync.dma_start(dkv0[:], dkv0_d.ap().rearrange("p (s c) -> p s c", s=9))
                dkv2 = p2.tile([128, 9, 128], bf, tag="dkv2", name="dkv2")
                nc.sync.dma_start(dkv2[:], dkv2_d.ap().rearrange("p (s c) -> p s c", s=9))
                dkv1v = p2.tile([64, 9, 64], bf, tag="dkv1v", name="dkv1v")
                nc.sync.dma_start(dkv1v[:], dkv1v_d.ap().rearrange("p (s c) -> p s c", s=9))

                kp0 = p2.tile([128, SR, SC], bf, tag="kp0", name="kp0")
                kp1v = p2.tile([64, SR, SC], bf, tag="kp1v", name="kp1v")
                kp2 = p2.tile([128, SR, SC], bf, tag="kp2", name="kp2")

                def kvpre_m(m, dsts, groups=G_K):
                    """1x1(fmap2) for kv channel chunk m; dsts: list of
                    (psum_p0, size, dst_tile, dst_p0)."""
                    for gi, (r0, rc) in enumerate(groups):
                        ps = psB.tile([128, rc * 66], f32, tag="b", name="psb")
                        for ki in range(2):
                            src = fmap2a if ki == 0 else F2B
                            mm(ps[:, :], kvw[ki][:, 128 * m:128 * (m + 1)],
                               src[0:(128 if ki == 0 else 64), r0:r0 + rc, 3:69],
                               ki == 0, ki == 1)
                        psv = ps[:, :].rearrange("p (r c) -> p r c", r=rc)
                        for ei, (p0, sz, dt_, d0) in enumerate(dsts):
                            d = dt_[d0:d0 + sz, r0:r0 + rc, 3:69]
                            s = psv[p0:p0 + sz, :, :]
                            if (gi + ei) % 2 == 0:
                                nc.scalar.copy(d, s)
                            else:
                                nc.vector.tensor_copy(d, s)

                kvpre_m(0, [(0, 128, kp0, 0)])
                kvpre_m(1, [(0, 64, F2B, 64), (64, 64, kp1v, 0)])

                q_sb = [p2.tile([128, LCORE], bf, tag="q_a", name="q_a"),
                        p2.tile([64, LCORE], bf, tag="q_b", name="q_b")]
                k_sb = [p2.tile([128, LCORE], bf, tag="k_a", name="k_a"),
                        p2.tile([64, LCORE], bf, tag="k_b", name="k_b")]
                v128 = p2.tile([128, LCORE], bf, tag="v128", name="v128")
                v64 = p2.tile([64, LCORE], bf, tag="v64", name="v64")

                ecnt = 0

                def dw_unit(diag, src, srcn, writes):
                    # weight-stationary: each shift's diag streams all four
                    # row-groups back-to-back so LDWEIGHTS amortizes
                    nonlocal ecnt
                    pst = [psW.tile([128, 512], f32, tag="w", name="psw")
                           for _ in range(4)]
                    for si in range(9):
                        dy, dx = si // 3, si % 3
                        w_ap = diag[:, si, :]
                        for t in range(4):
                            r0 = 4 + 8 * t + dy - 1
                            mm(pst[t][0:srcn, :], w_ap,
                               src[:, r0:r0 + 8, 3 + dx:3 + dx + 64],
                               si == 0, si == 8)
                    for t in range(4):
                        for (p0, sz, dst, d0) in writes:
                            d = dst[d0:d0 + sz, 512 * t:512 * (t + 1)]
                            s = pst[t][p0:p0 + sz, :]
                            if ecnt % 2 == 0:
                                nc.scalar.copy(d, s)
                            else:
                                nc.vector.tensor_copy(d, s)
                            ecnt += 1

                def transpose_t(src, dst, t):
                    pt = psW.tile([128, 128], bf, tag="w", name="pt")
                    nc.tensor.transpose(
                        pt[:, 0:128], src[0][:, 128 * t:128 * (t + 1)],
                        ident[:])
                    pt2 = psW.tile([128, 128], bf, tag="w", name="pt2")
                    nc.tensor.transpose(
                        pt2[:, 0:64], src[1][:, 128 * t:128 * (t + 1)],
                        ident[0:64, 0:64])
                    nc.vector.tensor_copy(dst[:, t, 0:128], pt[:, 0:128])
                    if dst is qT:
                        nc.vector.tensor_copy(dst[:, t, 128:192], pt2[:, 0:64])
                    else:
                        nc.scalar.copy(dst[:, t, 128:192], pt2[:, 0:64])

                qT = p2.tile([128, 16, DIM], bf, tag="qT", name="qT")
                kT = p2.tile([128, 16, DIM], bf, tag="kT", name="kT")

                # interleave psB-based kvpre m2 between psW units so PE keeps
                # streaming across unit-boundary PSUM recycling waits
                dw_unit(dqa, fmap2a, 128, [(0, 128, q_sb[0], 0)])
                kvpre_m(2, [(0, 128, kp2, 0)], groups=G_K[0:3])
                dw_unit(dqkb, F2B, 128,
                        [(0, 64, q_sb[1], 0), (64, 64, k_sb[1], 0)])
                kvpre_m(2, [(0, 128, kp2, 0)], groups=G_K[3:5])

                sq = p2.tile([128, LCORE], bf, tag="sq", name="sq")
                qss = [p2.tile([128, 1], f32, tag="qss_a", name="qss_a"),
                       p2.tile([64, 1], f32, tag="qss_b", name="qss_b")]
                kss = [p2.tile([128, 1], f32, tag="kss_a", name="kss_a"),
                       p2.tile([64, 1], f32, tag="kss_b", name="kss_b")]
                for ci in range(2):
                    n = 128 if ci == 0 else 64
                    nc.scalar.activation(sq[:n, :], q_sb[ci][:], AF.Square,
                                         accum_out=qss[ci][:])

                for t in range(16):
                    transpose_t(q_sb, qT, t)
                dw_unit(dkv0, kp0, 128, [(0, 128, k_sb[0], 0)])
                for ci in range(2):
                    n = 128 if ci == 0 else 64
                    nc.scalar.activation(sq[:n, :], k_sb[ci][:], AF.Square,
                                         accum_out=kss[ci][:])

                G2 = psB.tile([128, 384], f32, tag="b", name="G2")
                for t in range(16):
                    transpose_t(k_sb, kT, t)
                    mm(G2[:, 0:192], qT[:, t, 0:128], kT[:, t, 0:192],
                       t == 0, t == 15, skip=True)
                    mm(G2[0:64, 192:384], qT[:, t, 128:192], kT[:, t, 0:192],
                       t == 0, t == 15, skip=True)

                # staging tile for the paired AllReduce
                stag = p2.tile([128, 68], f32, tag="stag", name="stag")
                nc.gpsimd.memset(stag[:], 0.0)
                nc.vector.tensor_copy(stag[:, 0:1], qss[0][:])
                nc.vector.tensor_copy(stag[0:64, 1:2], qss[1][:])
                nc.vector.tensor_copy(stag[:, 2:3], kss[0][:])
                nc.vector.tensor_copy(stag[0:64, 3:4], kss[1][:])
                for h in range(4):
                    nc.vector.tensor_copy(
                        stag[32 * h:32 * h + 32, 4:36],
                        G2[32 * h:32 * h + 32, 32 * h:32 * h + 32])
                for h in range(2):
                    nc.vector.tensor_copy(
                        stag[32 * h:32 * h + 32, 36:68],
                        G2[32 * h:32 * h + 32, 320 + 32 * h:352 + 32 * h])

                arin = dramp.tile([128, 68], f32, tag="arin", name="arin")
                arout = dramp.tile([128, 68], f32, tag="arout", name="arout")
                nc.gpsimd.dma_start(arin[:], stag[:])
                nc.gpsimd.collective_compute(
                    "AllReduce", ALU.add, replica_groups=RG,
                    ins=[arin.opt()], outs=[arout.opt()])
                stagR = p2.tile([128, 68], f32, tag="stagR", name="stagR")
                nc.gpsimd.dma_start(stagR[:], arout[:])

                # v-side work hides under the collective
                dw_unit(dkv1v, kp1v, 64, [(0, 64, v128, 0)])
                dw_unit(dkv2, kp2, 128,
                        [(0, 64, v128, 64), (64, 64, v64, 0)])

                # inverse norms; temperature folded into q scale
                qsc = [p2.tile([128, 1], f32, tag="qsc_a", name="qsc_a"),
                       p2.tile([64, 1], f32, tag="qsc_b", name="qsc_b")]
                ksc = [p2.tile([128, 1], f32, tag="ksc_a", name="ksc_a"),
                       p2.tile([64, 1], f32, tag="ksc_b", name="ksc_b")]
                for ci in range(2):
                    n = 128 if ci == 0 else 64
                    nc.scalar.sqrt(qsc[ci][:], stagR[0:n, 0 + ci:1 + ci])
                    nc.vector.reciprocal(qsc[ci][:], qsc[ci][:])
                    nc.vector.tensor_mul(qsc[ci][:], qsc[ci][:],
                                         vecs[ci][:, VC_TEMP:VC_TEMP + 1])
                    nc.scalar.sqrt(ksc[ci][:], stagR[0:n, 2 + ci:3 + ci])
                    nc.vector.reciprocal(ksc[ci][:], ksc[ci][:])

                # per-head softmax into block-diagonal attn^T tiles.
                # logits = G*qinv*kinv*temp are bounded (|cos| <= temp), so
                # exp is computed without max subtraction.
                bdA = p2.tile([128, 128], bf, tag="bdA", name="bdA")
                bdB = p2.tile([64, 64], bf, tag="bdB", name="bdB")
                nc.gpsimd.memset(bdA[:], 0.0)
                nc.gpsimd.memset(bdB[:], 0.0)
                for h in range(HEADS):
                    ci, hb = (0, h) if h < 4 else (1, h - 4)
                    c0 = 32 * hb
                    if h < 4:
                        blk = stagR[c0:c0 + 32, 4:36]
                    else:
                        blk = stagR[c0:c0 + 32, 36:68]
                    s1 = p2.tile([32, 32], f32, tag="s1", bufs=3, name="s1")
                    nc.vector.tensor_scalar_mul(s1[:], blk, qsc[ci][c0:c0 + 32, :])
                    s2 = p2.tile([32, 32], f32, tag="s2", bufs=3, name="s2")
                    nc.vector.transpose(s2[:], s1[:])
                    nc.vector.tensor_scalar_mul(s2[:], s2[:], ksc[ci][c0:c0 + 32, :])
                    bd = bdA if h < 4 else bdB
                    nc.scalar.activation(bd[c0:c0 + 32, c0:c0 + 32], s2[:], AF.Exp)

                psum_s = psB.tile([128, 1], f32, tag="b", name="pss")
                mm(psum_s[0:128, :], bdA[:, :], onesb[:, :], True, True)
                psum_s2 = psB.tile([64, 1], f32, tag="b", name="pss2")
                mm(psum_s2[0:64, :], bdB[:, :], onesb[0:64, :], True, True)
                sinv = [p2.tile([128, 1], f32, tag="sinv_a", name="sinv_a"),
                        p2.tile([64, 1], f32, tag="sinv_b", name="sinv_b")]
                nc.vector.reciprocal(sinv[0][:], psum_s[:, :])
                nc.vector.reciprocal(sinv[1][:], psum_s2[:, :])

                o_sb = [p2.tile([128, LCORE], bf, tag="osb_a", name="osb_a"),
                        p2.tile([64, LCORE], bf, tag="osb_b", name="osb_b")]
                for t in range(4):
                    po = psB.tile([128, 512], f32, tag="b", name="po")
                    mm(po[:, :], bdA[:, :], v128[:, 512 * t:512 * (t + 1)],
                       True, True)
                    nc.vector.tensor_scalar_mul(
                        o_sb[0][:, 512 * t:512 * (t + 1)], po[:, :], sinv[0][:])
                    po2 = psB.tile([64, 512], f32, tag="b", name="po2")
                    mm(po2[:, :], bdB[:, :], v64[:, 512 * t:512 * (t + 1)],
                       True, True)
                    nc.scalar.activation(
                        o_sb[1][:, 512 * t:512 * (t + 1)], po2[:, :],
                        AF.Copy, scale=sinv[1][:])

                for mi, (m0, msz) in enumerate(CH):
                    for t in range(4):
                        ps = psB.tile([128, 512], f32, tag="b", name="psp")
                        for ki in range(2):
                            mm(ps[:msz, :], projw[ki][:, m0:m0 + msz],
                               o_sb[ki][:, 512 * t:512 * (t + 1)],
                               ki == 0, ki == 1)
                        st = p2.tile([128, 512], f32, tag="fo_st", bufs=3,
                                     name="fo_st")
                        if (mi * 4 + t) % 2 == 0:
                            nc.scalar.copy(st[:msz, :], ps[:msz, :])
                        else:
                            nc.vector.tensor_copy(st[:msz, :], ps[:msz, :])
                        nc.sync.dma_start(
                            out_d.ap()[m0:m0 + msz, 512 * t:512 * (t + 1)],
                            st[:msz, :])

    nc.compile()
    import os
    trace = bool(os.environ.get("KERNEL_TRACE"))
    res = run_bass_kernel_spmd(nc, host_inputs, core_ids=list(range(NCORES)),
                               trace=trace)
    global LAST_EXEC_NS
    LAST_EXEC_NS = res.exec_time_ns
    return res


LAST_EXEC_NS = None


def _prep_inputs(sel, inputs):
    """Build per-core in_maps (weights shared; xin slab + vecs per core)."""
    I = np.asarray(inputs["I"], dtype=np.float32)
    ex_ws = [np.asarray(inputs[f"ex_w{j}"], dtype=np.float32) for j in range(12)]
    ex_bs = [np.asarray(inputs[f"ex_b{j}"], dtype=np.float32) for j in range(12)]

    dense_slots = [i for i in range(TOPK) if GROUPS[sel[i]] == 1]
    dw1_slots = [i for i in range(TOPK) if GROUPS[sel[i]] == DIM and KSZ[sel[i]] == 1]
    pairs = _dw_pairs(sel)
    paired = {2 * j for (j, _) in pairs} | {2 * j + 1 for (j, _) in pairs}
    dw_solo = [i for i in range(TOPK)
               if GROUPS[sel[i]] == DIM and KSZ[sel[i]] > 1 and i not in paired]

    shared = {}
    for i in dense_slots:
        j = sel[i]
        ks = KSZ[j]
        kk = ks * ks
        w = ex_ws[j]  # [out, in, k, k]
        wa = w[:, 0:128].transpose(1, 2, 3, 0).reshape(128, kk, DIM)
        shared[f"e{i}_wa"] = np.ascontiguousarray(
            wa.reshape(128, kk * DIM)).astype(BF16)
        bsets = _b_pair_sets(ks)
        wp = np.zeros((128, len(bsets), DIM), dtype=np.float32)
        for bi, (dy, dx, dx2) in enumerate(bsets):
            wp[0:64, bi] = w[:, 128:192, dy, dx].T
            if dx2 is not None:
                wp[64:128, bi] = w[:, 128:192, dy, dx2].T
        shared[f"e{i}_wp"] = wp.reshape(128, len(bsets) * DIM).astype(BF16)

    for i in dw_solo + sorted(paired):
        j = sel[i]
        ks = KSZ[j]
        kk = ks * ks
        wv = ex_ws[j][:, 0, :, :].reshape(DIM, kk)
        da = np.zeros((128, kk, 128), dtype=np.float32)
        for c in range(128):
            da[c, :, c] = wv[c]
        shared[f"e{i}_da"] = da.reshape(128, kk * 128).astype(BF16)
        if i in dw_solo:
            db = np.zeros((64, kk, 64), dtype=np.float32)
            for c in range(64):
                db[c, :, c] = wv[128 + c]
            shared[f"e{i}_db"] = db.reshape(64, kk * 64).astype(BF16)
    for (jj, ks) in pairs:
        kk = ks * ks
        w0 = ex_ws[sel[2 * jj]][:, 0, :, :].reshape(DIM, kk)
        w1 = ex_ws[sel[2 * jj + 1]][:, 0, :, :].reshape(DIM, kk)
        dpb = np.zeros((128, kk, 128), dtype=np.float32)
        for c in range(64):
            dpb[c, :, c] = w0[128 + c]
            dpb[64 + c, :, 64 + c] = w1[128 + c]
        shared[f"e_pb{jj}"] = dpb.reshape(128, kk * 128).astype(BF16)

    exw = np.asarray(inputs["ex_out_w"], dtype=np.float32)  # [192, 768, 3, 3]
    wx = np.zeros((128, 6, 9, DIM), dtype=np.float32)
    for cc in range(4):
        blk = exw[:, 192 * cc:192 * cc + 128, :, :]
        wx[:, cc, :, :] = blk.transpose(1, 2, 3, 0).reshape(128, 9, DIM)
    for half in range(2):
        s0, s1 = 2 * half, 2 * half + 1
        b0 = exw[:, 192 * s0 + 128:192 * (s0 + 1), :, :]
        b1 = exw[:, 192 * s1 + 128:192 * (s1 + 1), :, :]
        wx[0:64, 4 + half] = b0.transpose(1, 2, 3, 0).reshape(64, 9, DIM)
        wx[64:128, 4 + half] = b1.transpose(1, 2, 3, 0).reshape(64, 9, DIM)
    shared["wexout"] = wx.reshape(128, 6 * 9 * DIM).astype(BF16)

    kvdw = np.asarray(inputs["kv_dw_w"], dtype=np.float32)[:, 0].reshape(384, 9)
    qdw = np.asarray(inputs["q_dw_w"], dtype=np.float32)[:, 0].reshape(DIM, 9)
    d0 = np.zeros((128, 9, 128), dtype=np.float32)
    d2 = np.zeros((128, 9, 128), dtype=np.float32)
    dqk = np.zeros((128, 9, 128), dtype=np.float32)
    d1v = np.zeros((64, 9, 64), dtype=np.float32)
    for c in range(128):
        d0[c, :, c] = kvdw[c]          # k channels 0..128
        d2[c, :, c] = kvdw[256 + c]    # v channels 64..192
    for c in range(64):
        dqk[c, :, c] = qdw[128 + c]            # q channels 128..192
        dqk[64 + c, :, 64 + c] = kvdw[128 + c]  # k channels 128..192
        d1v[c, :, c] = kvdw[192 + c]            # v channels 0..64
    shared["dkv0"] = d0.reshape(128, 9 * 128).astype(BF16)
    shared["dkv2"] = d2.reshape(128, 9 * 128).astype(BF16)
    shared["dqkb"] = dqk.reshape(128, 9 * 128).astype(BF16)
    shared["dkv1v"] = d1v.reshape(64, 9 * 64).astype(BF16)
    da = np.zeros((128, 9, 128), dtype=np.float32)
    for c in range(128):
        da[c, :, c] = qdw[c]
    shared["dq_a"] = da.reshape(128, 9 * 128).astype(BF16)

    kvw = np.asarray(inputs["kv_w"], dtype=np.float32)[:, :, 0, 0]
    shared["kvw"] = np.ascontiguousarray(kvw.T).astype(BF16)
    pw = np.asarray(inputs["proj_w"], dtype=np.float32)[:, :, 0, 0]
    shared["projw"] = np.ascontiguousarray(pw.T).astype(BF16)
    shared["ident"] = np.eye(128, dtype=np.float32).astype(BF16)
    shared["onesb"] = np.ones((128, 1), dtype=np.float32).astype(BF16)

    temp = np.asarray(inputs["temperature"], dtype=np.float32).reshape(HEADS)
    exob = np.asarray(inputs["ex_out_b"], dtype=np.float32)

    def build_vecs(mt, mb):
        v = np.zeros((DIM, VC_N), dtype=np.float32)
        for i in range(TOPK):
            if i in dw1_slots:
                continue
            v[:, VC_BIAS + i] = ex_bs[sel[i]]
            v[:, VC_BIAS_MT + i] = ex_bs[sel[i]] * mt
            v[:, VC_BIAS_MB + i] = ex_bs[sel[i]] * mb
        v[:, VC_XB] = exob
        v[:, VC_XB_MT] = exob * mt
        v[:, VC_XB_MB] = exob * mb
        v[:, VC_TEMP] = np.repeat(temp, DIM // HEADS)
        v[:, VC_MT] = mt
        v[:, VC_MB] = mb
        for sidx, i in enumerate(dw1_slots):
            base = VC_DW1 + 6 * sidx
            w = ex_ws[sel[i]][:, 0, 0, 0]
            b = ex_bs[sel[i]]
            v[:, base + 0] = w
            v[:, base + 1] = b
            v[:, base + 2] = w * mt
            v[:, base + 3] = b * mt
            v[:, base + 4] = w * mb
            v[:, base + 5] = b * mb
        return v

    def build_vecspb(mt, mb):
        v = np.zeros((128, 8), dtype=np.float32)
        v[:, VC_MT] = mt
        v[:, VC_MB] = mb
        for jj in range(2):
            for half in range(2):
                i = 2 * jj + half
                if i in dw1_slots:
                    continue  # dw1x1 path adds its own bias
                bb = ex_bs[sel[i]][128:192]
                r = slice(64 * half, 64 * half + 64)
                v[r, 3 * jj + 0] = bb
                v[r, 3 * jj + 1] = bb * mt
                v[r, 3 * jj + 2] = bb * mb
        return v

    vecs_h = [build_vecs(0.0, 1.0), build_vecs(1.0, 0.0)]
    vecspb_h = [build_vecspb(0.0, 1.0), build_vecspb(1.0, 0.0)]

    padded = np.zeros((B, DIM, 72, 72), dtype=np.float32)
    padded[:, :, 4:68, 4:68] = I
    in_maps = []
    for b in range(B):
        for h in range(2):
            m = dict(shared)
            slab = padded[b][:, 32 * h:32 * h + SR, :]
            m["xin"] = np.ascontiguousarray(slab).astype(BF16)
            if dense_slots:
                xb2s = np.zeros((128, SR, SC), dtype=np.float32)
                xb2s[0:64] = slab[128:192]
                xb2s[64:128, :, 0:SC - 1] = slab[128:192, :, 1:SC]
                m["xb2s"] = xb2s.astype(BF16)
            if pairs:
                xb2 = np.concatenate([slab[128:192], slab[128:192]], axis=0)
                m["xb2"] = np.ascontiguousarray(xb2).astype(BF16)
            m["vecs"] = vecs_h[h]
            m["vecspb"] = vecspb_h[h]
            in_maps.append(m)
    return in_maps


def kernel(**inputs) -> np.ndarray:
    I = np.asarray(inputs["I"], dtype=np.float32)
    T = np.asarray(inputs["T"], dtype=np.float32)
    pw = np.asarray(inputs["ca1_proj_w"], dtype=np.float32)
    sel = _select_experts(I, T, pw)
    in_maps = _prep_inputs(sel, inputs)
    res = _build_and_run(sel, in_maps)
    out = np.zeros((B, DIM, H, W), dtype=np.float32)
    for b in range(B):
        for h in range(2):
            out[b, :, 32 * h:32 * h + 32, :] = np.asarray(
                res.results[2 * b + h]["out"],
                dtype=np.float32).reshape(DIM, 32, 64)
    return out
